# revision 2
# baseline (speedup 1.0000x reference)
"""Trainium2 Bass kernel: batch-parallel LIF scan with PE-packed output.

Problem: x[128, 32, 8192] f32 -> spikes s[128, 32, 8192] f32 in {0, 1}.
    u' = u/2 + x_t ; s_t = (u' >= 1) ; u = u' * (1 - s_t)        (T=32 scan)

Sharding: data-parallel over batch (axis 0), 16 rows per NeuronCore x 8.
Per-core timestep slab [16, 8192] -> SBUF [128 partitions, 1024 free]
(partition p = b*8 + k, column n = k*1024 + f).

All columns run the RESCALED recurrence (z_t = 2^t u_t, bit-exact in fp32:
power-of-two scaling commutes with rounding):
    z' = z + 2^t x_t ; spike iff z' >= 2^t ; z = z' * (z' < 2^t)

Column lanes per step t (th = 2^t):
  DVE lane (V=760 cols, 2 sublanes so the serial chains pipeline):
      add   z' = (x mult th) add z        STT (prescale folded in)
      reset z  = (z' is_lt th) mult z'    fused STT (cmp+reset in one op)
  ACT:  g = Sign(z'/th - 1) -> bf16 {-1,+1}   spike bits for DVE cols
  Pool lane (264 cols): y = th*x (ACT Copy, in place); z' = z add y (TT);
      c = (z' is_lt th) -> bf16 {0,1} (TS, = pack bit AND reset mask);
      z = z' mult c (TT)

Output packing (PE, otherwise idle): the step bit-slab [128, 1024] bf16 is
multiplied by the diagonal stationary 2^(t%8) I (built once via gpsimd
affine_select) and accumulated in PSUM over 8-step groups; packs run one
step delayed so they never block in PE's in-order queue. Group sums
(integers, |q| <= 255, exact in bf16) are copied PSUM->SBUF bf16 by ACT
and stored from the SP queue: 4 x [128, 1024] bf16 = 1 MiB per core
instead of 4 MiB of per-step int8 stores, cutting the DMA bound.

Host decode: DVE cols s-bits = bits((q+255)/2); Pool cols s-bits = ~bits(q).
Verified bit-exact vs the jax reference on hardware (0/33.5M mismatches).
"""

import numpy as np

import concourse.bass as bass
import concourse.tile as tile
from concourse import bacc, mybir
from concourse.bass_utils import run_bass_kernel_spmd

B, T, N = 128, 32, 8192
NCORES = 8
BS = B // NCORES  # 16 batch rows per core
FW = 1024         # slab free width (8192 = 8 k-blocks * 1024)
GS = 8            # steps per pack group
NG = T // GS      # 4 groups

CFG = dict(
    mode="v5",
    dve_cols=760,   # V: columns on the DVE lane; rest on Pool
    dve_sub=2,      # DVE sublanes (independent chains hide sem latency)
    pool_sub=1,     # Pool sublanes
    xbufs=20,
    sbufs=6,
)

_cache = {}


def _build(cfg):
    V = cfg["dve_cols"]
    A = mybir.AluOpType
    F = mybir.ActivationFunctionType
    f32 = mybir.dt.float32
    bf16 = mybir.dt.bfloat16

    nc = bacc.Bacc("TRN2", target_bir_lowering=False, debug=False,
                   num_devices=NCORES)
    x = nc.dram_tensor("x", [BS, T, N], f32, kind="ExternalInput").ap()
    q = nc.dram_tensor("q", [NG, BS, N], bf16, kind="ExternalOutput").ap()
    xr = x.rearrange("b t (k f) -> t b k f", f=FW)
    qr = q.rearrange("g b (k f) -> g b k f", f=FW)

    with tile.TileContext(nc) as tc:
        with (
            tc.tile_pool(name="xp", bufs=cfg["xbufs"]) as xpool,
            tc.tile_pool(name="sp", bufs=cfg["sbufs"]) as spool,
            tc.tile_pool(name="qp", bufs=2) as qpool,
            tc.tile_pool(name="up", bufs=1) as upool,
            tc.tile_pool(name="pp", bufs=1, space=bass.MemorySpace.PSUM) as ppool,
        ):
            # --- one-time setup (hidden under the first DMA load) -------
            bias = upool.tile([128, 1], f32, tag="bias")
            nc.vector.memset(bias[:], -1.0)
            zB = upool.tile([128, FW], f32, tag="zB")
            nc.vector.memset(zB[:, :V], 0.0)
            nc.gpsimd.memset(zB[:, V:], 0.0)
            zA = [upool.tile([128, FW], f32, tag=f"zA{i}", name=f"zA{i}")
                  for i in range(2)]
            # 8 diagonal stationaries 2^k * I, bf16 [128, 128]
            wI = []
            for k in range(GS):
                w = upool.tile([128, 128], bf16, tag=f"wI{k}", name=f"wI{k}")
                nc.gpsimd.memset(w[:], float(2 ** k))
                nc.gpsimd.affine_select(
                    out=w[:], in_=w[:], pattern=[[1, 128]],
                    compare_op=A.is_equal, fill=0.0,
                    base=0, channel_multiplier=-1)
                wI.append(w)
            # PSUM accumulators: 2 regions (group parity) x 2 halves
            qp = [[ppool.tile([128, 512], f32, tag=f"qp{r}{h}",
                              name=f"qp{r}{h}") for h in range(2)]
                  for r in range(2)]

            # sublane column ranges: DVE sublanes split [0, V); Pool [V, FW)
            dsub = []
            for j in range(cfg["dve_sub"]):
                lo = V * j // cfg["dve_sub"]
                hi = V * (j + 1) // cfg["dve_sub"]
                dsub.append((lo, hi))
            psub = []
            for j in range(cfg["pool_sub"]):
                lo = V + (FW - V) * j // cfg["pool_sub"]
                hi = V + (FW - V) * (j + 1) // cfg["pool_sub"]
                psub.append((lo, hi))

            slabs = []

            def emit_pack(tp):
                kp = tp % GS
                gp = tp // GS
                slp = slabs[tp]
                for h in range(2):
                    nc.tensor.matmul(
                        qp[gp % 2][h][:], wI[kp][:],
                        slp[:, h * 512:(h + 1) * 512],
                        start=(kp == 0), stop=(kp == GS - 1))
                if kp == GS - 1:
                    qs = qpool.tile([128, FW], bf16, tag="q", name="qs")
                    for h in range(2):
                        lo, hi = h * 512, (h + 1) * 512
                        nc.scalar.activation(qs[:, lo:hi], qp[gp % 2][h][:],
                                             F.Copy, bias=0.0, scale=1.0)
                        nc.sync.dma_start(out=qr[gp][:, :, lo:hi],
                                          in_=qs[:, lo:hi])

            for t in range(T):
                th = float(2.0 ** t)
                k = t % GS
                grp = t // GS
                a = zA[t % 2]
                xt = xpool.tile([128, FW], f32, tag="x", name="xt")
                if t == 0:
                    nc.sync.dma_start(out=xt[:, :512], in_=xr[t][:, :, :512])
                    nc.sync.dma_start(out=xt[:, 512:], in_=xr[t][:, :, 512:])
                else:
                    nc.sync.dma_start(out=xt[:], in_=xr[t])

                # Pool lane prescale: y = 2^t * x  (in place, ACT)
                nc.scalar.activation(xt[:, V:], xt[:, V:], F.Copy,
                                     bias=0.0, scale=th)
                # integrate (adds per sublane unless add_wide)
                if cfg.get("add_wide"):
                    nc.vector.scalar_tensor_tensor(
                        out=a[:, :V], in0=xt[:, :V], scalar=th,
                        in1=zB[:, :V], op0=A.mult, op1=A.add)
                else:
                    for lo, hi in dsub:
                        nc.vector.scalar_tensor_tensor(
                            out=a[:, lo:hi], in0=xt[:, lo:hi], scalar=th,
                            in1=zB[:, lo:hi], op0=A.mult, op1=A.add)
                for lo, hi in psub:
                    nc.gpsimd.tensor_tensor(
                        out=a[:, lo:hi], in0=xt[:, lo:hi], in1=zB[:, lo:hi],
                        op=A.add)

                # DVE fused reset first in list order (keeps its wait off
                # the ACT Sign); Pool lane needs its c bits before its reset
                sl = spool.tile([128, FW], bf16, tag="s", name="sl")
                for lo, hi in dsub:
                    nc.vector.scalar_tensor_tensor(
                        out=zB[:, lo:hi], in0=a[:, lo:hi], scalar=th,
                        in1=a[:, lo:hi], op0=A.is_lt, op1=A.mult)
                for lo, hi in psub:
                    nc.gpsimd.tensor_scalar(
                        out=sl[:, lo:hi], in0=a[:, lo:hi], scalar1=th,
                        scalar2=None, op0=A.is_lt)
                for lo, hi in psub:
                    nc.gpsimd.tensor_tensor(
                        out=zB[:, lo:hi], in0=a[:, lo:hi], in1=sl[:, lo:hi],
                        op=A.mult)
                nc.scalar.activation(sl[:, :V], a[:, :V], F.Sign,
                                     bias=bias[:], scale=1.0 / th)

                # pack one step delayed (bits long ready -> the pack never
                # blocks in PE's in-order queue)
                slabs.append(sl)
                if t > 0:
                    emit_pack(t - 1)
            emit_pack(T - 1)
    nc.compile()
    return nc


def _get_nc(cfg=None):
    cfg = dict(CFG if cfg is None else cfg)
    key = tuple(sorted(cfg.items()))
    if key not in _cache:
        _cache[key] = (_build(cfg), cfg)
    return _cache[key]


def _postprocess(qv: np.ndarray, cfg) -> np.ndarray:
    """qv: [B, NG, BS... wait — concatenated [NG, BS, N] per core stacked on
    batch: full [NCORES * NG? ...] see run(). Input here: [NG, B, N] after
    per-core gather. Decode to spikes [B, T, N] f32."""
    V = cfg["dve_cols"]
    qf = qv.astype(np.float32)                    # [NG, B, N]
    ncol = np.arange(N) % FW
    is_v = (ncol < V)[None, None, :]
    S = np.where(is_v, (qf + 255.0) / 2.0, 255.0 - qf)
    S = S.astype(np.int32)                        # [NG, B, N] in [0, 255]
    ks = np.arange(GS, dtype=np.int32)
    bits = (S[:, :, :, None] >> ks[None, None, None, :]) & 1   # [NG,B,N,GS]
    out = bits.transpose(1, 0, 3, 2).reshape(-1, T, N)          # [B, T, N]
    return out.astype(np.float32)


def run(x: np.ndarray, cfg=None, trace: bool = False):
    nc, cfg = _get_nc(cfg)
    in_maps = [{"x": np.ascontiguousarray(x[c * BS:(c + 1) * BS])}
               for c in range(NCORES)]
    res = run_bass_kernel_spmd(nc, in_maps, list(range(NCORES)), trace=trace)
    qv = np.concatenate([np.asarray(res.results[c]["q"])[:, None]
                         for c in range(NCORES)], axis=1)  # [NG, NCORES, BS, N]
    qv = qv.reshape(NG, B, N)
    return _postprocess(qv, cfg), res


def kernel(x: np.ndarray) -> np.ndarray:
    out, _ = run(np.asarray(x))
    return out


# revision 3
# speedup vs baseline: 1.0117x; 1.0117x over previous
"""Trainium2 Bass kernel: batch-parallel LIF scan with PE-packed output.

Problem: x[128, 32, 8192] f32 -> spikes s[128, 32, 8192] f32 in {0, 1}.
    u' = u/2 + x_t ; s_t = (u' >= 1) ; u = u' * (1 - s_t)        (T=32 scan)

Sharding: data-parallel over batch (axis 0), 16 rows per NeuronCore x 8.
Per-core timestep slab [16, 8192] -> SBUF [128 partitions, 1024 free]
(partition p = b*8 + k, column n = k*1024 + f).

All columns run the RESCALED recurrence (z_t = 2^t u_t, bit-exact in fp32:
power-of-two scaling commutes with rounding):
    z' = z + 2^t x_t ; spike iff z' >= 2^t ; z = z' * (z' < 2^t)

Column lanes per step t (th = 2^t):
  DVE lane (V=760 cols, 2 sublanes so the serial chains pipeline):
      add   z' = (x mult th) add z        STT (prescale folded in)
      reset z  = (z' is_lt th) mult z'    fused STT (cmp+reset in one op)
  ACT:  g = Sign(z'/th - 1) -> bf16 {-1,+1}   spike bits for DVE cols
  Pool lane (264 cols): y = th*x (ACT Copy, in place); z' = z add y (TT);
      c = (z' is_lt th) -> bf16 {0,1} (TS, = pack bit AND reset mask);
      z = z' mult c (TT)

Output packing (PE, otherwise idle): the step bit-slab [128, 1024] bf16 is
multiplied by the diagonal stationary 2^(t%8) I (built once via gpsimd
affine_select) and accumulated in PSUM over 8-step groups; packs run one
step delayed so they never block in PE's in-order queue. Group sums
(integers, |q| <= 255, exact in bf16) are copied PSUM->SBUF bf16 by ACT
and stored from the SP queue: 4 x [128, 1024] bf16 = 1 MiB per core
instead of 4 MiB of per-step int8 stores, cutting the DMA bound.

Host decode: DVE cols s-bits = bits((q+255)/2); Pool cols s-bits = ~bits(q).
Verified bit-exact vs the jax reference on hardware (0/33.5M mismatches).
"""

import numpy as np

import concourse.bass as bass
import concourse.tile as tile
from concourse import bacc, mybir
from concourse.bass_utils import run_bass_kernel_spmd

B, T, N = 128, 32, 8192
NCORES = 8
BS = B // NCORES  # 16 batch rows per core
FW = 1024         # slab free width (8192 = 8 k-blocks * 1024)
GS = 8            # steps per pack group
NG = T // GS      # 4 groups

CFG = dict(
    mode="v5",
    dve_cols=768,   # V: columns on the DVE lane; rest on Pool
    dve_sub=2,      # DVE sublanes (independent chains hide sem latency)
    pool_sub=1,     # Pool sublanes
    xbufs=20,
    sbufs=6,
)

_cache = {}


def _build(cfg):
    V = cfg["dve_cols"]
    A = mybir.AluOpType
    F = mybir.ActivationFunctionType
    f32 = mybir.dt.float32
    bf16 = mybir.dt.bfloat16

    nc = bacc.Bacc("TRN2", target_bir_lowering=False, debug=False,
                   num_devices=NCORES)
    x = nc.dram_tensor("x", [BS, T, N], f32, kind="ExternalInput").ap()
    q = nc.dram_tensor("q", [NG, BS, N], bf16, kind="ExternalOutput").ap()
    xr = x.rearrange("b t (k f) -> t b k f", f=FW)
    qr = q.rearrange("g b (k f) -> g b k f", f=FW)

    with tile.TileContext(nc) as tc:
        with (
            tc.tile_pool(name="xp", bufs=cfg["xbufs"]) as xpool,
            tc.tile_pool(name="sp", bufs=cfg["sbufs"]) as spool,
            tc.tile_pool(name="qp", bufs=2) as qpool,
            tc.tile_pool(name="up", bufs=1) as upool,
            tc.tile_pool(name="pp", bufs=1, space=bass.MemorySpace.PSUM) as ppool,
        ):
            # --- one-time setup (hidden under the first DMA load) -------
            bias = upool.tile([128, 1], f32, tag="bias")
            nc.vector.memset(bias[:], -1.0)
            zB = upool.tile([128, FW], f32, tag="zB")
            nc.vector.memset(zB[:, :V], 0.0)
            nc.gpsimd.memset(zB[:, V:], 0.0)
            zA = [upool.tile([128, FW], f32, tag=f"zA{i}", name=f"zA{i}")
                  for i in range(2)]
            # 8 diagonal stationaries 2^k * I, bf16 [128, 128]
            wI = []
            for k in range(GS):
                w = upool.tile([128, 128], bf16, tag=f"wI{k}", name=f"wI{k}")
                nc.gpsimd.memset(w[:], float(2 ** k))
                nc.gpsimd.affine_select(
                    out=w[:], in_=w[:], pattern=[[1, 128]],
                    compare_op=A.is_equal, fill=0.0,
                    base=0, channel_multiplier=-1)
                wI.append(w)
            # PSUM accumulators: 2 regions (group parity) x 2 halves
            qp = [[ppool.tile([128, 512], f32, tag=f"qp{r}{h}",
                              name=f"qp{r}{h}") for h in range(2)]
                  for r in range(2)]

            # sublane column ranges: DVE sublanes split [0, V); Pool [V, FW)
            dsub = []
            for j in range(cfg["dve_sub"]):
                lo = V * j // cfg["dve_sub"]
                hi = V * (j + 1) // cfg["dve_sub"]
                dsub.append((lo, hi))
            psub = []
            for j in range(cfg["pool_sub"]):
                lo = V + (FW - V) * j // cfg["pool_sub"]
                hi = V + (FW - V) * (j + 1) // cfg["pool_sub"]
                psub.append((lo, hi))

            slabs = []

            def emit_pack(tp):
                kp = tp % GS
                gp = tp // GS
                slp = slabs[tp]
                for h in range(2):
                    nc.tensor.matmul(
                        qp[gp % 2][h][:], wI[kp][:],
                        slp[:, h * 512:(h + 1) * 512],
                        start=(kp == 0), stop=(kp == GS - 1))
                if kp == GS - 1:
                    qs = qpool.tile([128, FW], bf16, tag="q", name="qs")
                    for h in range(2):
                        lo, hi = h * 512, (h + 1) * 512
                        nc.scalar.activation(qs[:, lo:hi], qp[gp % 2][h][:],
                                             F.Copy, bias=0.0, scale=1.0)
                        nc.sync.dma_start(out=qr[gp][:, :, lo:hi],
                                          in_=qs[:, lo:hi])

            for t in range(T):
                th = float(2.0 ** t)
                k = t % GS
                grp = t // GS
                a = zA[t % 2]
                xt = xpool.tile([128, FW], f32, tag="x", name="xt")
                if t == 0:
                    nc.sync.dma_start(out=xt[:, :512], in_=xr[t][:, :, :512])
                    nc.sync.dma_start(out=xt[:, 512:], in_=xr[t][:, :, 512:])
                else:
                    nc.sync.dma_start(out=xt[:], in_=xr[t])

                # Pool lane prescale: y = 2^t * x  (in place, ACT)
                nc.scalar.activation(xt[:, V:], xt[:, V:], F.Copy,
                                     bias=0.0, scale=th)
                # integrate (adds per sublane unless add_wide)
                if cfg.get("add_wide"):
                    nc.vector.scalar_tensor_tensor(
                        out=a[:, :V], in0=xt[:, :V], scalar=th,
                        in1=zB[:, :V], op0=A.mult, op1=A.add)
                else:
                    for lo, hi in dsub:
                        nc.vector.scalar_tensor_tensor(
                            out=a[:, lo:hi], in0=xt[:, lo:hi], scalar=th,
                            in1=zB[:, lo:hi], op0=A.mult, op1=A.add)
                for lo, hi in psub:
                    nc.gpsimd.tensor_tensor(
                        out=a[:, lo:hi], in0=xt[:, lo:hi], in1=zB[:, lo:hi],
                        op=A.add)

                # DVE fused reset first in list order (keeps its wait off
                # the ACT Sign); Pool lane needs its c bits before its reset
                sl = spool.tile([128, FW], bf16, tag="s", name="sl")
                for lo, hi in dsub:
                    nc.vector.scalar_tensor_tensor(
                        out=zB[:, lo:hi], in0=a[:, lo:hi], scalar=th,
                        in1=a[:, lo:hi], op0=A.is_lt, op1=A.mult)
                for lo, hi in psub:
                    nc.gpsimd.tensor_scalar(
                        out=sl[:, lo:hi], in0=a[:, lo:hi], scalar1=th,
                        scalar2=None, op0=A.is_lt)
                for lo, hi in psub:
                    nc.gpsimd.tensor_tensor(
                        out=zB[:, lo:hi], in0=a[:, lo:hi], in1=sl[:, lo:hi],
                        op=A.mult)
                nc.scalar.activation(sl[:, :V], a[:, :V], F.Sign,
                                     bias=bias[:], scale=1.0 / th)

                # pack one step delayed (bits long ready -> the pack never
                # blocks in PE's in-order queue)
                slabs.append(sl)
                if t > 0:
                    emit_pack(t - 1)
            emit_pack(T - 1)
    nc.compile()
    return nc


def _get_nc(cfg=None):
    cfg = dict(CFG if cfg is None else cfg)
    key = tuple(sorted(cfg.items()))
    if key not in _cache:
        _cache[key] = (_build(cfg), cfg)
    return _cache[key]


def _postprocess(qv: np.ndarray, cfg) -> np.ndarray:
    """qv: [B, NG, BS... wait — concatenated [NG, BS, N] per core stacked on
    batch: full [NCORES * NG? ...] see run(). Input here: [NG, B, N] after
    per-core gather. Decode to spikes [B, T, N] f32."""
    V = cfg["dve_cols"]
    qf = qv.astype(np.float32)                    # [NG, B, N]
    ncol = np.arange(N) % FW
    is_v = (ncol < V)[None, None, :]
    S = np.where(is_v, (qf + 255.0) / 2.0, 255.0 - qf)
    S = S.astype(np.int32)                        # [NG, B, N] in [0, 255]
    ks = np.arange(GS, dtype=np.int32)
    bits = (S[:, :, :, None] >> ks[None, None, None, :]) & 1   # [NG,B,N,GS]
    out = bits.transpose(1, 0, 3, 2).reshape(-1, T, N)          # [B, T, N]
    return out.astype(np.float32)


def run(x: np.ndarray, cfg=None, trace: bool = False):
    nc, cfg = _get_nc(cfg)
    in_maps = [{"x": np.ascontiguousarray(x[c * BS:(c + 1) * BS])}
               for c in range(NCORES)]
    res = run_bass_kernel_spmd(nc, in_maps, list(range(NCORES)), trace=trace)
    qv = np.concatenate([np.asarray(res.results[c]["q"])[:, None]
                         for c in range(NCORES)], axis=1)  # [NG, NCORES, BS, N]
    qv = qv.reshape(NG, B, N)
    return _postprocess(qv, cfg), res


def kernel(x: np.ndarray) -> np.ndarray:
    out, _ = run(np.asarray(x))
    return out
